# revision 9
# baseline (speedup 1.0000x reference)
"""Trainium2 Bass kernel for Conf-MPU loss (nn_Conf_MPULoss).

Strategy: the loss is a streaming reduction over N rows x 5 classes down to a
handful of per-class accumulators, followed by a trivial scalar combination.

Host side:
  - rows are partitioned by label t into 5 class groups (stable), split evenly
    across 8 cores, each per-core class segment padded to S = 128*R rows.
  - pivot transform: for a segment with label c the host sends the 4 columns
    y_i = x_i - x_c (i != c) as bf16, class-major blocks of width R. This
    both (a) drops DMA from 20B/row fp32 to 8B/row bf16 and (b) makes
    Zc1 := 1 + sum_i e^{y_i} = Z / e^{x_c} = 1/p_c, so the per-row reciprocal
    1/p_c needed by the conf term is free (no divide, no second exp).
  - the C-length accumulators from all cores are reduced on host (fp64) and
    combined into the final scalar.

Device side (per core, SPMD over 8 cores), per class segment [128, 4R] bf16
(blocks b=0..3 of width R; for c<4 block 3 is z4 = x4 - xc):
    E    = exp(Y)                      ScalarE (one LUT set: exp+ln)
    T1   = E[:,0:2R] + E[:,2R:4R]      DVE TT bf16 (2x mode, dense)
    Zc1  = (T1a + 1) + T1b             DVE STT bf16
    LnZ  = ln(Zc1)                     ScalarE, accum-> sum(ln) [c<4]
    c < 4 (LnZ = dt = -log p_c):
      D4 = LnZ - z4 (= -log p_neg)     DVE STT, accum-> sum(D4)
      U  = Zc1 * D4                    GpSimd TT
      num: (Zc1 < 2) * U, accum        DVE STT   [p_c > 0.5 <=> Zc1 < 2]
      den: (Zc1 < 2), accum            DVE tensor_scalar (4x mode)
      (sd_c = sum(z4) = sum(ln) - sum(D4), pads removed exactly on host)
    c == 4, two half-tiles (LnZ = d4 = -log p_neg):
      T2 = max(Ea, Eb); EM = max(..)   GpSimd TT
      W4 = 2*EM - Zc1                  DVE STT   [all pos p<=0.5 <=> W4<=0]
      M  = (Zc1 >= 2) * LnZ            DVE STT   [p_neg<=0.5 <=> Zc1>=2]
      li: (W4 <= 0) * M, accum         DVE STT

Sentinel pads: +20 in all 4 cols for c<4 (mask off, z4=20 exactly corrected
on host), -20 for the negative segment (Zc1=1 -> everything 0).
"""

import numpy as np
import ml_dtypes

import concourse.bacc as bacc
import concourse.mybir as mybir
import concourse.tile as tile
from concourse import bass_utils

F32 = mybir.dt.float32
BF16 = mybir.dt.bfloat16
Alu = mybir.AluOpType
Act = mybir.ActivationFunctionType

P = 128
NCLS = 5
N_CORES = 8
NCOLS = 18  # 4 pos classes * [num, den, sumD4, sumLn] + [liA, liB]

BF = ml_dtypes.bfloat16

_PROGRAM_CACHE: dict[int, object] = {}


def _restrict_act_tables(arch: str):
    """Confine Exp/Ln to the natural_log_exp_and_others set so the act-table
    pass emits a single ACT_TABLE_LOAD instead of thrashing between sets."""
    from concourse import hw_specs

    tables = hw_specs.get_activation_tables(arch)
    if "natural_log_exp_and_others" not in tables:
        return
    for name, funcs in tables.items():
        if name != "natural_log_exp_and_others":
            funcs.discard(Act.Exp)
            funcs.discard(Act.Ln)


def _build_program(R: int):
    """Build + compile the per-core Bass program for segment width 4R."""
    assert R % 2 == 0
    H = R // 2  # half-tile block width for the negative segment
    nc = bacc.Bacc("TRN2", debug=False, num_devices=N_CORES)
    _restrict_act_tables(nc.m.arch)
    y_d = nc.dram_tensor("y", [NCLS, P, 4 * R], BF16, kind="ExternalInput").ap()
    st_d = nc.dram_tensor("stats", [P, NCOLS], F32, kind="ExternalOutput").ap()

    with tile.TileContext(nc) as tc:
        with (
            tc.tile_pool(name="io", bufs=6) as iop,
            tc.tile_pool(name="ep", bufs=3) as epp,
            tc.tile_pool(name="wk", bufs=3) as wp,
            tc.tile_pool(name="st", bufs=1) as sp,
        ):
            stats = sp.tile([P, NCOLS], F32)
            nc.vector.memset(stats, 0.0)
            ONES = sp.tile([P, R], BF16)
            nc.vector.memset(ONES, 1.0)

            # ---- negative segment first (longest chain), as two half-tiles
            # so the pipeline primes early. Z' = sum(e^y); Zp1 = Z'+1 folded
            # into the Ln bias and the compare thresholds. ----
            for h in range(2):
                W = 2 * R * h  # column offset of this half in y_d[4]
                Y = iop.tile([P, 2 * R], BF16, tag=f"y4{h}")
                nc.sync.dma_start(out=Y, in_=y_d[4][:, W : W + 2 * R])
                E = epp.tile([P, 2 * R], BF16, tag="e4")
                nc.scalar.activation(E, Y, Act.Exp)
                T1 = wp.tile([P, R], BF16, tag="t1")
                nc.vector.tensor_tensor(
                    out=T1, in0=E[:, 0:R], in1=E[:, R : 2 * R], op=Alu.add
                )
                Z = wp.tile([P, H], BF16, tag="z")
                nc.vector.tensor_tensor(
                    out=Z, in0=T1[:, 0:H], in1=T1[:, H:R], op=Alu.add
                )
                # D4 = -log p_neg = ln(Z' + 1)
                D4 = wp.tile([P, H], BF16, tag="lnz")
                nc.scalar.activation(D4, Z, Act.Ln, bias=1.0)
                T2 = wp.tile([P, R], BF16, tag="t2")
                nc.vector.tensor_tensor(
                    out=T2, in0=E[:, 0:R], in1=E[:, R : 2 * R], op=Alu.max
                )
                EM = wp.tile([P, H], BF16, tag="em")
                nc.vector.tensor_tensor(
                    out=EM, in0=T2[:, 0:H], in1=T2[:, H:R], op=Alu.max
                )
                # V = 2*EM - Z'; all positive p <= 0.5  <=>  V <= 1
                E2 = wp.tile([P, H], BF16, tag="e2")
                nc.gpsimd.tensor_scalar(
                    out=E2, in0=EM, scalar1=2.0, scalar2=0.0,
                    op0=Alu.mult, op1=Alu.add,
                )
                V = wp.tile([P, H], BF16, tag="v")
                nc.gpsimd.tensor_tensor(out=V, in0=E2, in1=Z, op=Alu.subtract)
                # M = (p_neg <= 0.5) * D4  <=>  (Z' >= 1) * D4
                M = wp.tile([P, H], BF16, tag="m")
                nc.vector.scalar_tensor_tensor(
                    out=M, in0=Z, scalar=1.0, in1=D4,
                    op0=Alu.is_ge, op1=Alu.mult,
                )
                G = wp.tile([P, H], BF16, tag="g")
                nc.vector.scalar_tensor_tensor(
                    out=G, in0=V, scalar=1.0, in1=M,
                    op0=Alu.is_le, op1=Alu.mult,
                    accum_out=stats[:, 16 + h : 17 + h],
                )

            # ---- positive segments c = 0..3 ----
            for c in range(4):
                Y = iop.tile([P, 4 * R], BF16, tag="y")
                nc.sync.dma_start(out=Y, in_=y_d[c])
                E = epp.tile([P, 4 * R], BF16, tag="e")
                nc.scalar.activation(E, Y, Act.Exp)
                T1 = wp.tile([P, 2 * R], BF16, tag="t1")
                nc.vector.tensor_tensor(
                    out=T1, in0=E[:, 0 : 2 * R], in1=E[:, 2 * R : 4 * R], op=Alu.add
                )
                Z = wp.tile([P, R], BF16, tag="z")
                nc.vector.tensor_tensor(
                    out=Z, in0=T1[:, 0:R], in1=T1[:, R : 2 * R], op=Alu.add
                )
                # dt = -log p_c = ln(Z' + 1); accum -> sum(dt)
                DT = wp.tile([P, R], BF16, tag="lnz")
                nc.scalar.activation(
                    DT, Z, Act.Ln, bias=1.0,
                    accum_out=stats[:, 4 * c + 3 : 4 * c + 4],
                )
                # D4 = -log p_neg = dt - z4; accum -> sum(dt - z4)
                D4 = wp.tile([P, R], BF16, tag="d4")
                nc.vector.scalar_tensor_tensor(
                    out=D4,
                    in0=DT,
                    scalar=1.0,
                    in1=Y[:, 3 * R : 4 * R],
                    op0=Alu.mult,
                    op1=Alu.subtract,
                    accum_out=stats[:, 4 * c + 2 : 4 * c + 3],
                )
                # U = (1/p_c) * D4 = (Z' + 1) * D4
                Z1 = wp.tile([P, R], BF16, tag="z1")
                nc.gpsimd.tensor_scalar(
                    out=Z1, in0=Z, scalar1=1.0, scalar2=0.0,
                    op0=Alu.add, op1=Alu.add,
                )
                U = wp.tile([P, R], BF16, tag="u")
                nc.gpsimd.tensor_tensor(out=U, in0=Z1, in1=D4, op=Alu.mult)
                # p_c > 0.5  <=>  Z' < 1
                G = wp.tile([P, R], BF16, tag="g")
                nc.vector.scalar_tensor_tensor(
                    out=G,
                    in0=Z,
                    scalar=1.0,
                    in1=U,
                    op0=Alu.is_lt,
                    op1=Alu.mult,
                    accum_out=stats[:, 4 * c : 4 * c + 1],
                )
                Gd = wp.tile([P, R], BF16, tag="gd")
                nc.vector.scalar_tensor_tensor(
                    out=Gd,
                    in0=Z,
                    scalar=1.0,
                    in1=ONES,
                    op0=Alu.is_lt,
                    op1=Alu.mult,
                    accum_out=stats[:, 4 * c + 1 : 4 * c + 2],
                )
            nc.sync.dma_start(out=st_d, in_=stats)
    nc.compile()
    return nc


def _get_program(R: int):
    if R not in _PROGRAM_CACHE:
        _PROGRAM_CACHE[R] = _build_program(R)
    return _PROGRAM_CACHE[R]


def _prepare_inputs(x: np.ndarray, t: np.ndarray):
    """Sort rows by class, pivot-transform, shard across cores, pad segments.
    Returns (in_maps, counts, n_pad, R)."""
    N = x.shape[0]
    t64 = t.astype(np.int64, copy=False)
    counts = np.bincount(t64, minlength=NCLS).astype(np.int64)

    # per-core per-class row counts (even split of each class across cores)
    n_ck = np.zeros((NCLS, N_CORES), dtype=np.int64)
    for c in range(NCLS):
        q, r = divmod(int(counts[c]), N_CORES)
        n_ck[c] = q
        n_ck[c, :r] += 1

    R = int(max(8, -(-int(n_ck.max()) // P)))
    R = (R + 1) // 2 * 2  # even: negative segment splits into two halves
    S = P * R

    order = np.argsort(t64, kind="stable")
    xs = np.ascontiguousarray(x[order], dtype=np.float32)
    starts = np.concatenate([[0], np.cumsum(counts)])

    ycores = np.empty((N_CORES, NCLS, P, 4 * R), dtype=BF)
    for c in range(NCLS):
        off = int(starts[c])
        cols = [i for i in range(NCLS) if i != c]
        if c < 4:
            cols = [i for i in cols if i != 4] + [4]  # z4 in the last block
        pad = np.float32(20.0 if c < 4 else -20.0)
        for k in range(N_CORES):
            n = int(n_ck[c, k])
            ys = np.full((S, 4), pad, dtype=np.float32)
            if n:
                seg = xs[off : off + n]
                ys[:n] = seg[:, cols] - seg[:, c : c + 1]
                off += n
            if c < 4:
                # [S, 4] -> [128, 4R] class-major blocks of width R
                ycores[k, c] = (
                    ys.reshape(P, R, 4).transpose(0, 2, 1).reshape(P, 4 * R)
                )
            else:
                # two half-tiles, each [128, 4H] with H = R//2
                H = R // 2
                for h in range(2):
                    half = ys[h * S // 2 : (h + 1) * S // 2]
                    ycores[k, c, :, h * 2 * R : (h + 1) * 2 * R] = (
                        half.reshape(P, H, 4).transpose(0, 2, 1).reshape(P, 4 * H)
                    )

    in_maps = [{"y": ycores[k]} for k in range(N_CORES)]
    n_pad = N_CORES * S - counts  # per class, summed over cores
    return in_maps, counts, n_pad, R


def _combine(stats_list, counts, n_pad, N, R):
    """Host all-reduce of the C-length accumulators + final scalar combination."""
    st = np.zeros(NCOLS, dtype=np.float64)
    for s in stats_list:
        st += s.astype(np.float64).sum(axis=0)

    counts = counts.astype(np.float64)
    # exact per-pad contribution to sum_ln - sum_d4, replicating device math:
    # pad rows are +20 in all 4 cols; the Ln accum taps pre-bf16-round fp32
    # while D4 subtracts the bf16-rounded dt, so the pad residual is
    # dt_f32 - bf16(dt_f32) + 20 (z4_pad = 20 is bf16-exact).
    e = np.float32(np.exp(np.float32(20.0))).astype(BF).astype(np.float32)
    t1 = (e + e).astype(BF).astype(np.float32)
    zp = (t1 + t1).astype(BF).astype(np.float32)
    dtf = np.float64(np.log1p(np.float64(zp)))
    dtb = np.float64(np.float32(dtf).astype(BF).astype(np.float64))
    pad_res = (dtf - dtb) + 20.0

    r13 = 0.0  # risk1 - risk3
    r2 = 0.0
    for c in range(4):
        num = st[4 * c + 0]
        den = st[4 * c + 1]
        sum_d4 = st[4 * c + 2]
        sum_ln = st[4 * c + 3]
        sd = (sum_ln - sum_d4) - pad_res * float(n_pad[c])  # sum_{t=c}(x4 - xc)
        prior = counts[c] / N
        r13 += prior * sd / max(1.0, counts[c])
        r2 += prior * num / max(den, 1.0)
    li = st[16] + st[17]
    r4 = li / max(1.0, counts[4])

    pos = 4.0 * (r13 + r2)
    if pos < 0.0:
        pos = 0.0
    return np.float32(pos + r4)


def run_device(in_maps, R, trace=False, **kw):
    nc = _get_program(R)
    res = bass_utils.run_bass_kernel_spmd(
        nc, in_maps, core_ids=list(range(N_CORES)), trace=trace, **kw
    )
    return res


def kernel(x: np.ndarray, t: np.ndarray) -> np.ndarray:
    x = np.asarray(x, dtype=np.float32)
    t = np.asarray(t)
    N = x.shape[0]
    in_maps, counts, n_pad, R = _prepare_inputs(x, t)
    res = run_device(in_maps, R)
    stats_list = [res.results[k]["stats"] for k in range(N_CORES)]
    return _combine(stats_list, counts, n_pad, N, R)


# revision 11
# speedup vs baseline: 1.9411x; 1.9411x over previous
"""Trainium2 Bass kernel for Conf-MPU loss (nn_Conf_MPULoss).

Strategy: the loss is a streaming reduction over N rows x 5 classes down to a
handful of per-class accumulators, followed by a trivial scalar combination.

Host side:
  - rows are partitioned by label t into 5 class groups (stable), split evenly
    across 8 cores, each per-core class segment padded to S = 128*R rows.
  - pivot transform: for a segment with label c the host sends the 4 columns
    y_i = x_i - x_c (i != c) as bf16, class-major blocks of width R. This
    both (a) drops DMA from 20B/row fp32 to 8B/row bf16 and (b) makes
    Zc1 := 1 + sum_i e^{y_i} = Z / e^{x_c} = 1/p_c, so the per-row reciprocal
    1/p_c needed by the conf term is free (no divide, no second exp).
  - the C-length accumulators from all cores are reduced on host (fp64) and
    combined into the final scalar.

Device side (per core, SPMD over 8 cores), per class segment [128, 4R] bf16
(blocks b=0..3 of width R; for c<4 block 3 is z4 = x4 - xc):
    E    = exp(Y)                      ScalarE (one LUT set: exp+ln)
    T1   = E[:,0:2R] + E[:,2R:4R]      DVE TT bf16 (2x mode, dense)
    Zc1  = (T1a + 1) + T1b             DVE STT bf16
    LnZ  = ln(Zc1)                     ScalarE, accum-> sum(ln) [c<4]
    c < 4 (LnZ = dt = -log p_c):
      D4 = LnZ - z4 (= -log p_neg)     DVE STT, accum-> sum(D4)
      U  = Zc1 * D4                    GpSimd TT
      num: (Zc1 < 2) * U, accum        DVE STT   [p_c > 0.5 <=> Zc1 < 2]
      den: (Zc1 < 2), accum            DVE tensor_scalar (4x mode)
      (sd_c = sum(z4) = sum(ln) - sum(D4), pads removed exactly on host)
    c == 4, two half-tiles (LnZ = d4 = -log p_neg):
      T2 = max(Ea, Eb); EM = max(..)   GpSimd TT
      W4 = 2*EM - Zc1                  DVE STT   [all pos p<=0.5 <=> W4<=0]
      M  = (Zc1 >= 2) * LnZ            DVE STT   [p_neg<=0.5 <=> Zc1>=2]
      li: (W4 <= 0) * M, accum         DVE STT

Sentinel pads: +20 in all 4 cols for c<4 (mask off, z4=20 exactly corrected
on host), -20 for the negative segment (Zc1=1 -> everything 0).
"""

import numpy as np
import ml_dtypes

import concourse.bacc as bacc
import concourse.mybir as mybir
import concourse.tile as tile
from concourse import bass_utils

F32 = mybir.dt.float32
BF16 = mybir.dt.bfloat16
Alu = mybir.AluOpType
Act = mybir.ActivationFunctionType

P = 128
NCLS = 5
N_CORES = 8
NCOLS = 18  # 4 pos classes * [num, den, sumD4, sumLn] + [liA, liB]

BF = ml_dtypes.bfloat16

_PROGRAM_CACHE: dict[int, object] = {}


def _restrict_act_tables(arch: str):
    """Confine Exp/Ln to the natural_log_exp_and_others set so the act-table
    pass emits a single ACT_TABLE_LOAD instead of thrashing between sets."""
    from concourse import hw_specs

    tables = hw_specs.get_activation_tables(arch)
    if "natural_log_exp_and_others" not in tables:
        return
    for name, funcs in tables.items():
        if name != "natural_log_exp_and_others":
            funcs.discard(Act.Exp)
            funcs.discard(Act.Ln)


def _build_program(R: int):
    """Build + compile the per-core Bass program for segment width 4R."""
    assert R % 2 == 0
    H = R // 2  # half-tile block width for the negative segment
    nc = bacc.Bacc("TRN2", debug=False, num_devices=N_CORES)
    _restrict_act_tables(nc.m.arch)
    y_d = nc.dram_tensor("y", [NCLS, P, 4 * R], BF16, kind="ExternalInput").ap()
    st_d = nc.dram_tensor("stats", [P, NCOLS], F32, kind="ExternalOutput").ap()

    with tile.TileContext(nc) as tc:
        with (
            tc.tile_pool(name="io", bufs=6) as iop,
            tc.tile_pool(name="ep", bufs=3) as epp,
            tc.tile_pool(name="wk", bufs=3) as wp,
            tc.tile_pool(name="st", bufs=1) as sp,
        ):
            stats = sp.tile([P, NCOLS], F32)
            nc.vector.memset(stats, 0.0)
            ONES = sp.tile([P, R], BF16)
            nc.vector.memset(ONES, 1.0)

            # ---- negative segment first (longest chain), as two half-tiles
            # so the pipeline primes early. Z' = sum(e^y); Zp1 = Z'+1 folded
            # into the Ln bias and the compare thresholds. ----
            for h in range(2):
                W = 2 * R * h  # column offset of this half in y_d[4]
                Y = iop.tile([P, 2 * R], BF16, tag=f"y4{h}")
                nc.sync.dma_start(out=Y, in_=y_d[4][:, W : W + 2 * R])
                E = epp.tile([P, 2 * R], BF16, tag="e4")
                nc.scalar.activation(E, Y, Act.Exp)
                T1 = wp.tile([P, R], BF16, tag="t1")
                nc.vector.tensor_tensor(
                    out=T1, in0=E[:, 0:R], in1=E[:, R : 2 * R], op=Alu.add
                )
                Z = wp.tile([P, H], BF16, tag="z")
                nc.vector.tensor_tensor(
                    out=Z, in0=T1[:, 0:H], in1=T1[:, H:R], op=Alu.add
                )
                # D4 = -log p_neg = ln(Z' + 1)
                D4 = wp.tile([P, H], BF16, tag="lnz")
                nc.scalar.activation(D4, Z, Act.Ln, bias=1.0)
                T2 = wp.tile([P, R], BF16, tag="t2")
                nc.vector.tensor_tensor(
                    out=T2, in0=E[:, 0:R], in1=E[:, R : 2 * R], op=Alu.max
                )
                EM = wp.tile([P, H], BF16, tag="em")
                nc.vector.tensor_tensor(
                    out=EM, in0=T2[:, 0:H], in1=T2[:, H:R], op=Alu.max
                )
                # W4 = 2*EM - Z'; all positive p <= 0.5  <=>  W4 <= 1
                W4 = wp.tile([P, H], BF16, tag="w4")
                nc.vector.scalar_tensor_tensor(
                    out=W4, in0=EM, scalar=2.0, in1=Z,
                    op0=Alu.mult, op1=Alu.subtract,
                )
                # M = (p_neg <= 0.5) * D4  <=>  (Z' >= 1) * D4
                M = wp.tile([P, H], BF16, tag="m")
                nc.vector.scalar_tensor_tensor(
                    out=M, in0=Z, scalar=1.0, in1=D4,
                    op0=Alu.is_ge, op1=Alu.mult,
                )
                G = wp.tile([P, H], BF16, tag="g")
                nc.vector.scalar_tensor_tensor(
                    out=G, in0=W4, scalar=1.0, in1=M,
                    op0=Alu.is_le, op1=Alu.mult,
                    accum_out=stats[:, 16 + h : 17 + h],
                )

            # ---- positive segments c = 0..3 ----
            for c in range(4):
                Y = iop.tile([P, 4 * R], BF16, tag="y")
                nc.sync.dma_start(out=Y, in_=y_d[c])
                E = epp.tile([P, 4 * R], BF16, tag="e")
                nc.scalar.activation(E, Y, Act.Exp)
                T1 = wp.tile([P, 2 * R], BF16, tag="t1")
                nc.vector.tensor_tensor(
                    out=T1, in0=E[:, 0 : 2 * R], in1=E[:, 2 * R : 4 * R], op=Alu.add
                )
                Z = wp.tile([P, R], BF16, tag="z")
                nc.vector.tensor_tensor(
                    out=Z, in0=T1[:, 0:R], in1=T1[:, R : 2 * R], op=Alu.add
                )
                # dt = -log p_c = ln(Z' + 1); accum -> sum(dt)
                DT = wp.tile([P, R], BF16, tag="lnz")
                nc.scalar.activation(
                    DT, Z, Act.Ln, bias=1.0,
                    accum_out=stats[:, 4 * c + 3 : 4 * c + 4],
                )
                # D4 = -log p_neg = dt - z4; accum -> sum(dt - z4)
                D4 = wp.tile([P, R], BF16, tag="d4")
                nc.vector.scalar_tensor_tensor(
                    out=D4,
                    in0=DT,
                    scalar=1.0,
                    in1=Y[:, 3 * R : 4 * R],
                    op0=Alu.mult,
                    op1=Alu.subtract,
                    accum_out=stats[:, 4 * c + 2 : 4 * c + 3],
                )
                # U = (1/p_c) * D4 = (Z' + 1) * D4; +1 via 1-op ts (4x mode)
                Z1 = wp.tile([P, R], BF16, tag="z1")
                nc.vector.tensor_scalar(
                    out=Z1, in0=Z, scalar1=1.0, scalar2=None, op0=Alu.add
                )
                U = wp.tile([P, R], BF16, tag="u")
                nc.gpsimd.tensor_tensor(out=U, in0=Z1, in1=D4, op=Alu.mult)
                # p_c > 0.5  <=>  Z' < 1
                G = wp.tile([P, R], BF16, tag="g")
                nc.vector.scalar_tensor_tensor(
                    out=G,
                    in0=Z,
                    scalar=1.0,
                    in1=U,
                    op0=Alu.is_lt,
                    op1=Alu.mult,
                    accum_out=stats[:, 4 * c : 4 * c + 1],
                )
                if c < 2:
                    # den on DVE
                    Gd = wp.tile([P, R], BF16, tag="gd")
                    nc.vector.scalar_tensor_tensor(
                        out=Gd,
                        in0=Z,
                        scalar=1.0,
                        in1=ONES,
                        op0=Alu.is_lt,
                        op1=Alu.mult,
                        accum_out=stats[:, 4 * c + 1 : 4 * c + 2],
                    )
                else:
                    # den on ScalarE: sum(sign(1 - Z')) -> den = (acc + S)/2
                    Gs = wp.tile([P, R], BF16, tag="gs")
                    nc.scalar.activation(
                        Gs, Z, Act.Sign, scale=-1.0, bias=1.0,
                        accum_out=stats[:, 4 * c + 1 : 4 * c + 2],
                    )
            nc.sync.dma_start(out=st_d, in_=stats)
    nc.compile()
    return nc


def _get_program(R: int):
    if R not in _PROGRAM_CACHE:
        _PROGRAM_CACHE[R] = _build_program(R)
    return _PROGRAM_CACHE[R]


def _prepare_inputs(x: np.ndarray, t: np.ndarray):
    """Sort rows by class, pivot-transform, shard across cores, pad segments.
    Returns (in_maps, counts, n_pad, R)."""
    N = x.shape[0]
    t64 = t.astype(np.int64, copy=False)
    counts = np.bincount(t64, minlength=NCLS).astype(np.int64)

    # per-core per-class row counts (even split of each class across cores)
    n_ck = np.zeros((NCLS, N_CORES), dtype=np.int64)
    for c in range(NCLS):
        q, r = divmod(int(counts[c]), N_CORES)
        n_ck[c] = q
        n_ck[c, :r] += 1

    R = int(max(8, -(-int(n_ck.max()) // P)))
    R = (R + 1) // 2 * 2  # even: negative segment splits into two halves
    S = P * R

    order = np.argsort(t64, kind="stable")
    xs = np.ascontiguousarray(x[order], dtype=np.float32)
    starts = np.concatenate([[0], np.cumsum(counts)])

    ycores = np.empty((N_CORES, NCLS, P, 4 * R), dtype=BF)
    for c in range(NCLS):
        off = int(starts[c])
        cols = [i for i in range(NCLS) if i != c]
        if c < 4:
            cols = [i for i in cols if i != 4] + [4]  # z4 in the last block
        pad = np.float32(20.0 if c < 4 else -20.0)
        for k in range(N_CORES):
            n = int(n_ck[c, k])
            ys = np.full((S, 4), pad, dtype=np.float32)
            if n:
                seg = xs[off : off + n]
                ys[:n] = seg[:, cols] - seg[:, c : c + 1]
                off += n
            if c < 4:
                # [S, 4] -> [128, 4R] class-major blocks of width R
                ycores[k, c] = (
                    ys.reshape(P, R, 4).transpose(0, 2, 1).reshape(P, 4 * R)
                )
            else:
                # two half-tiles, each [128, 4H] with H = R//2
                H = R // 2
                for h in range(2):
                    half = ys[h * S // 2 : (h + 1) * S // 2]
                    ycores[k, c, :, h * 2 * R : (h + 1) * 2 * R] = (
                        half.reshape(P, H, 4).transpose(0, 2, 1).reshape(P, 4 * H)
                    )

    in_maps = [{"y": ycores[k]} for k in range(N_CORES)]
    n_pad = N_CORES * S - counts  # per class, summed over cores
    return in_maps, counts, n_pad, R


def _combine(stats_list, counts, n_pad, N, R):
    """Host all-reduce of the C-length accumulators + final scalar combination."""
    st = np.zeros(NCOLS, dtype=np.float64)
    for s in stats_list:
        st += s.astype(np.float64).sum(axis=0)

    counts = counts.astype(np.float64)
    # exact per-pad contribution to sum_ln - sum_d4, replicating device math:
    # pad rows are +20 in all 4 cols; the Ln accum taps pre-bf16-round fp32
    # while D4 subtracts the bf16-rounded dt, so the pad residual is
    # dt_f32 - bf16(dt_f32) + 20 (z4_pad = 20 is bf16-exact).
    e = np.float32(np.exp(np.float32(20.0))).astype(BF).astype(np.float32)
    t1 = (e + e).astype(BF).astype(np.float32)
    zp = (t1 + t1).astype(BF).astype(np.float32)
    dtf = np.float64(np.log1p(np.float64(zp)))
    dtb = np.float64(np.float32(dtf).astype(BF).astype(np.float64))
    pad_res = (dtf - dtb) + 20.0

    r13 = 0.0  # risk1 - risk3
    r2 = 0.0
    S_total = float(N_CORES * P * R)  # rows per class segment across cores
    for c in range(4):
        num = st[4 * c + 0]
        den = st[4 * c + 1]
        if c >= 2:
            den = (den + S_total) / 2.0  # sign-sum -> count
        sum_d4 = st[4 * c + 2]
        sum_ln = st[4 * c + 3]
        sd = (sum_ln - sum_d4) - pad_res * float(n_pad[c])  # sum_{t=c}(x4 - xc)
        prior = counts[c] / N
        r13 += prior * sd / max(1.0, counts[c])
        r2 += prior * num / max(den, 1.0)
    li = st[16] + st[17]
    r4 = li / max(1.0, counts[4])

    pos = 4.0 * (r13 + r2)
    if pos < 0.0:
        pos = 0.0
    return np.float32(pos + r4)


def run_device(in_maps, R, trace=False, **kw):
    nc = _get_program(R)
    res = bass_utils.run_bass_kernel_spmd(
        nc, in_maps, core_ids=list(range(N_CORES)), trace=trace, **kw
    )
    return res


def kernel(x: np.ndarray, t: np.ndarray) -> np.ndarray:
    x = np.asarray(x, dtype=np.float32)
    t = np.asarray(t)
    N = x.shape[0]
    in_maps, counts, n_pad, R = _prepare_inputs(x, t)
    res = run_device(in_maps, R)
    stats_list = [res.results[k]["stats"] for k in range(N_CORES)]
    return _combine(stats_list, counts, n_pad, N, R)
